# revision 25
# baseline (speedup 1.0000x reference)
"""CenterFormer bbox head as a fused 3-stage matmul chain on 8 TRN2 cores.

Reference computation (per batch b, per proposal n):
  y = relu(BN(shared_w @ x + shared_b))            # 256 -> 64
  h = relu(BN(heads_w1[h] @ y + heads_b1[h]))      # 64 -> 64, 6 heads
  o = heads_w2[h] @ h + heads_b2[h]                # 64 -> 3 (padded), slice+concat -> 12

Host-side preprocessing folds BN (eval mode) into the conv weights, stacks the
6 head convs into a single [384, 64] matmul, builds a block-diagonal
[12, 384] final conv that directly emits the channel-concatenated output, and
casts x + weights to the matmul dtype (default fp16) so no on-device casts are
needed and the input DMA moves half the bytes.

Sharding: data-parallel over batch: core b handles ct_feat[b] ([256, 16384]).

PE-array packing (tile_position row/col groups run concurrently on TRN2):
tiles are processed in pairs (2j, 2j+1). Stage 1: the pair's [64,F] outputs
occupy the two col-halves of ONE py PSUM bank; interleaved chunk matmuls
stream concurrently and a single fused [128,F] relu evacuates both (b1
host-duplicated to 128 rows). Stage 2 (K=64): chunk m for both tiles runs
as a concurrent row-tile pair (T0 rows 0-63 / T8 rows 64-127 of the shared
ys tile; W2^T duplicated host-side into partitions 64-127), landing in the
two bank-halves of a double-bank [128,2F] PSUM tile evacuated by a single
cross-bank relu (same b2[m] for both halves). Stage 3 (M=12): the pair
shares one po bank at col offsets 0/32, chunk matmuls adjacent ->
concurrent; ONE fused op (+bias at rows 0-11/32-43) evacuates both slices
into [44, 2048] block staging, shipped by 2 strided DMAs per 8-tile block.
Effective PE streams drop 8 -> 4 per tile; ACT/DVE PSUM evacuation
(greedily balanced; Pool cannot read PSUM) is the co-roof. A deep software
skew (s1 pair at 2j, s2 pair at 2j+3/+4, s3 pair at 2j+5) keeps every
producer >1.5us ahead of its consumer.

All constants are packed into two tensors loaded with one DMA each (weights in
the matmul dtype, biases in f32), and tiny warm-up ops make PE/ACT observe
those DMAs up front: a self-loading fp32 Matmult only has one sync-wait slot
in walrus codegen, so no matmul may ever need to wait on two semaphores.
"""

import numpy as np

BN_EPS = 1e-3
HEAD_CH = (3, 2, 1, 3, 2, 1)
B, CIN, N, CS, HN = 8, 256, 16384, 64, 6
COUT = sum(HEAD_CH)  # 12
NCORES = 8

# matmul dtype: fp16 streams at full PE rate (1 cyc/row); x and weights are
# cast host-side (10 mantissa bits: ~3e-4 rel err, vs 2e-2 tolerance)
MM_DTYPE = "f16"

F = 512    # matmul free-dim tile (one fp32 PSUM bank)
FD = 4096  # staging width
FDMA = 1024  # input DMA chunk width (descriptors stay >= 512B in fp16)

# packed weight-tile column offsets: w1 [128,128] | w2 [64,384] | w3 [128,36]
W1_OFF, W2_OFF, W3_OFF, W_COLS = 0, 128, 512, 548
# packed bias-tile column offsets: b1 [64,1] | b2 [128,3] | b3 [12,1]
B1_OFF, B2_OFF, B3_OFF, B_COLS = 0, 1, 4, 5

_CACHE: dict = {}


def _build_bass(mm_dtype: str, repeat: int = 1, unroll: int = 1):
    import concourse.bacc as bacc
    import concourse.mybir as mybir
    from concourse.tile import TileContext

    f32 = mybir.dt.float32
    # f32r: stream fp32 bits through the PE in float32r mode (full rate at
    # free-dim >= 256, vs 4 cycles/row for plain fp32). Same 4-byte layout;
    # matmul operands and their producers carry the float32r dtype.
    mdt = {"f32": f32, "f32r": mybir.dt.float32r, "bf16": mybir.dt.bfloat16,
           "f16": mybir.dt.float16}[mm_dtype]
    AF = mybir.ActivationFunctionType
    r = lambda ap: ap

    # Bacc (not raw Bass): its finalize() runs move_matmul_waits_to_ldweights
    # + generate_event_semaphores, which split multi-sem waits that walrus
    # codegen rejects ("Too many sync wait commands").
    nc = bacc.Bacc()
    x = nc.declare_dram_parameter("x", [CIN, N], mdt, isOutput=False)
    wp = nc.declare_dram_parameter("wp", [128, W_COLS], mdt, isOutput=False)
    bp = nc.declare_dram_parameter("bp", [128, B_COLS], f32, isOutput=False)
    out = nc.declare_dram_parameter("out", [COUT, N], f32, isOutput=True)

    with TileContext(nc) as tc:
        with (
            tc.tile_pool(name="const", bufs=1) as cpool,
            tc.tile_pool(name="xin", bufs=3) as xpool,
            tc.tile_pool(name="acts", bufs=8) as apool,
            tc.tile_pool(name="outs", bufs=4) as opool,
            tc.tile_pool(name="psum", bufs=2, space="PSUM") as ppool,
        ):
            wt = cpool.tile([128, W_COLS], mdt)
            nc.scalar.dma_start(out=wt[:], in_=wp[:])
            bt = cpool.tile([128, B_COLS], f32)
            nc.scalar.dma_start(out=bt[:], in_=bp[:])

            w1 = wt[:, W1_OFF : W1_OFF + 128]          # stage-1 lhsT, 2 K-chunks
            w2 = wt[:64, W2_OFF : W2_OFF + 384]        # stage-2 lhsT (T0)
            w2b = wt[64:128, W2_OFF : W2_OFF + 384]    # stage-2 lhsT (T8)
            w3 = wt[:, W3_OFF : W3_OFF + 36]           # stage-3 lhsT, 3 K-chunks
            b1p = bt[:, B1_OFF : B1_OFF + 1]   # b1 duplicated to 128 rows
            b3p = bt[:44, B3_OFF : B3_OFF + 1]  # b3 at rows 0-11 and 32-43

            # Warm-ups: make PE/ACT/DVE/Pool observe the const DMAs via
            # single-wait ops so no later matmul needs a second sync-wait slot.
            pw = ppool.tile([1, 1], f32, tag="py", bufs=1)
            wwu = (wt[:, 0:1].bitcast(f32) if mm_dtype == "f32r"
                   else wt[:, 0:1])
            nc.tensor.matmul(pw[:], wwu, wwu, start=True, stop=True)
            sw = apool.tile([1, 4], f32, tag="warm")
            nc.scalar.activation(sw[:, 0:1], bt[0:1, 0:1], AF.Copy)
            nc.vector.tensor_copy(sw[:, 1:2], bt[0:1, 0:1])

            xr = x.rearrange("(k p) n -> p k n", p=128)

            # benchmarking: wrap the whole pass in a HW loop (repeat > 1)
            import contextlib
            loop_cm = (tc.For_i(0, repeat,
                                hint_engines=(mybir.EngineType.PE,))
                       if repeat > 1 else contextlib.nullcontext())

            # Greedy elementwise load-balancer across ACT / DVE. (Pool/GPSIMD
            # cannot read PSUM — BIR verifier rejects it — so it sits out.)
            # Costs (ns, cost-model v2 at F=512, PSUM f32 in): activation
            # ~612, DVE tensor_scalar ~658.
            est = {"ACT": 0.0, "DVE": 0.0}
            cost = {"ACT": 612.0, "DVE": 658.0}

            def relu_bias(dst, src, bias_ap, w=None):
                # per-engine wide-op weights: DVE's slower cycle (1.042 vs
                # 0.833 ns) makes [128,2F] ops relatively pricier there
                w = w or {"ACT": 1.0, "DVE": 1.0}
                eng = min(est, key=lambda e: est[e] + w[e] * cost[e])
                est[eng] += w[eng] * cost[eng]
                if eng == "ACT":
                    nc.scalar.activation(dst, src, AF.Relu, bias=bias_ap)
                else:
                    nc.vector.tensor_scalar(dst, src, bias_ap, 0.0,
                                            mybir.AluOpType.add,
                                            mybir.AluOpType.max)

            def add_bias(dst, src, bias_ap):
                eng = min(est, key=lambda e: est[e] + cost[e])
                est[eng] += cost[eng]
                if eng == "ACT":
                    nc.scalar.activation(dst, src, AF.Identity, bias=bias_ap)
                else:
                    nc.vector.tensor_scalar(dst, src, bias_ap, None,
                                            mybir.AluOpType.add)

            # Software-pipelined flat tile loop with 2-tile skew: iteration t
            # emits stage-1 for tile t, stage-2 for t-1, stage-3 for t-2, so
            # every relu has ~5-8 matmuls (>1.7us) of PE work as slack before
            # its consumer and PE never stalls on ACT/DVE/Pool latency.
            NT = N // F          # 32 tiles
            TPB = FD // F        # 8 tiles per staging block
            xts, ots, yss, hss, pys = {}, {}, {}, {}, {}

            def emit_s1_pair(j):
                # tiles 2j / 2j+1 write the two col-halves of one py bank;
                # interleaved chunk matmuls stream concurrently, and one
                # fused [128,F] relu evacuates both (b1 is host-duplicated
                # into partitions 64-127).
                ta, tb = 2 * j, 2 * j + 1
                i, jja = divmod(ta, TPB)
                if jja == 0:
                    xt = xpool.tile([128, 2, FD], mdt, tag="xt")
                    for c in range(FD // FDMA):
                        cs = slice(c * FDMA, (c + 1) * FDMA)
                        nc.sync.dma_start(
                            out=xt[:, :, cs],
                            in_=xr[:, :, i * FD + c * FDMA
                                   : i * FD + (c + 1) * FDMA])
                    xts[i] = xt
                xt = xts[i]
                pys[j] = ppool.tile([128, F], f32, tag="py", name="py",
                                    bufs=1)
                yss[j] = apool.tile([128, F], mdt, tag="ys", bufs=3,
                                    name="ys")
                for k in range(2):
                    for half, t in ((0, ta), (1, tb)):
                        sl = slice((t % TPB) * F, (t % TPB) * F + F)
                        nc.tensor.matmul(
                            pys[j][64 * half : 64 * half + 64, :],
                            r(w1[:, 64 * k : 64 * k + 64]),
                            r(xt[:, k, sl]),
                            start=(k == 0), stop=(k == 1))
                relu_bias(yss[j][:], pys[j][:], b1p)

            def emit_s2_pair(j, chunks):
                # chunk m for both pair tiles lands in the two bank-halves of
                # ONE double-bank PSUM tile; a single [128, 2F] relu (same
                # b2[m] bias for both halves) evacuates it.
                ys = yss[j]
                for m in chunks:
                    ph2 = ppool.tile([128, 2 * F], f32, tag="ph", bufs=3,
                                     name="ph2")
                    for half, wsel in ((0, w2), (1, w2b)):
                        nc.tensor.matmul(
                            ph2[:, half * F : (half + 1) * F],
                            r(wsel[:, m * 128 : (m + 1) * 128]),
                            r(ys[64 * half : 64 * half + 64, :]),
                            start=True, stop=True)
                    hs2 = apool.tile([128, 2 * F], mdt, tag="hs", bufs=6,
                                     name="hs2")
                    relu_bias(hs2[:], ph2[:],
                              bt[:, B2_OFF + m : B2_OFF + m + 1],
                              w={"ACT": 1.70, "DVE": 1.91})
                    for half, t in ((0, 2 * j), (1, 2 * j + 1)):
                        hss.setdefault(t, [None, None, None])[m] = \
                            hs2[:, half * F : (half + 1) * F]
                if chunks and chunks[-1] == 2:
                    yss.pop(j)

            def emit_s3_pair(j):
                # tiles 2j / 2j+1 share one po bank at col offsets 0 / 32;
                # chunk-m matmuls for the two tiles are adjacent col-tiles
                # and stream concurrently.
                ta, tb = 2 * j, 2 * j + 1
                po = ppool.tile([128, F], f32, tag="po", name="po",
                                bufs=1)
                for m in range(3):
                    for half, t in ((0, ta), (1, tb)):
                        nc.tensor.matmul(
                            po[32 * half : 32 * half + COUT, :],
                            r(w3[:, m * 12 : (m + 1) * 12]),
                            r(hss[t][m]), start=(m == 0), stop=(m == 2))
                hss.pop(ta), hss.pop(tb)
                # one fused op evacuates BOTH tiles' slices (+bias) into a
                # [44, 2048] block staging tile; rows 12-31 are garbage and
                # never leave SBUF. 2 strided DMAs per 8-tile block.
                i, p = divmod(j, TPB // 2)
                if p == 0:
                    ots[i] = opool.tile([44, FD // 2], f32, tag="ot",
                                        name="ot")
                add_bias(ots[i][:, p * F : (p + 1) * F], po[0:44, :], b3p)
                if p == TPB // 2 - 1:
                    ot = ots.pop(i)
                    ob = out[:, i * FD : (i + 1) * FD].rearrange(
                        "r (p f) -> r p f", f=2 * F)
                    ov = ot.rearrange("r (p f) -> r p f", f=F)
                    nc.scalar.dma_start(out=ob[:, :, 0:F],
                                        in_=ov[0:COUT, :, :])
                    nc.scalar.dma_start(out=ob[:, :, F : 2 * F],
                                        in_=ov[32 : 32 + COUT, :, :])

            if repeat < 0:  # sim-only: python-unroll |repeat| passes
                unroll, loop_cm = -repeat, contextlib.nullcontext()
            with loop_cm:
                for _ in range(unroll):
                    for t in range(NT + 6):
                        if t < NT and t % 2 == 0:
                            emit_s1_pair(t // 2)
                        if t >= 3 and (t - 3) % 2 == 0 and (t - 3) // 2 < NT // 2:
                            emit_s2_pair((t - 3) // 2, [0, 1])
                        if t >= 4 and (t - 4) % 2 == 0 and (t - 4) // 2 < NT // 2:
                            emit_s2_pair((t - 4) // 2, [2])
                        if t >= 5 and (t - 5) % 2 == 0 and (t - 5) // 2 < NT // 2:
                            emit_s3_pair((t - 5) // 2)

    nc.finalize()  # runs Bacc.compile(): wait-splitting, reg-alloc, DCE
    _check_matmul_waits(nc)
    return nc


def _check_matmul_waits(nc):
    import concourse.mybir as mybir

    bad = []
    for f in nc.m.functions:
        for blk in f.blocks:
            for inst in blk.instructions:
                if isinstance(inst, mybir.InstMatmult) and inst.sync_info:
                    if len(inst.sync_info.on_wait) > 1:
                        bad.append((inst.name,
                                    [w.ant_name for w in inst.sync_info.on_wait]))
    if bad:
        raise RuntimeError(f"matmuls with >1 sync wait (walrus limit): {bad}")


def _get_nc(mm_dtype: str, repeat: int = 1, unroll: int = 1):
    key = (mm_dtype, repeat, unroll)
    if key not in _CACHE:
        _CACHE[key] = _build_bass(mm_dtype, repeat, unroll)
    return _CACHE[key]


def _np_mm_dtype(mm_dtype: str):
    if mm_dtype == "bf16":
        import ml_dtypes
        return ml_dtypes.bfloat16
    if mm_dtype == "f16":
        return np.float16
    return np.float32


def _fold_params(inputs, mm_dtype: str):
    """Fold BN into conv weights; pack into the on-device tile layouts."""
    f = lambda k: np.asarray(inputs[k], np.float32)

    inv1 = f("shared_gamma") / np.sqrt(f("shared_var") + BN_EPS)          # [64]
    W1 = f("shared_w") * inv1[:, None]                                    # [64, 256]
    b1v = f("shared_b") * inv1 + f("shared_beta") - f("shared_mean") * inv1

    inv2 = f("heads_gamma") / np.sqrt(f("heads_var") + BN_EPS)            # [6, 64]
    W2 = (f("heads_w1") * inv2[:, :, None]).reshape(HN * CS, CS)          # [384, 64]
    b2v = (f("heads_b1") * inv2 + f("heads_beta")
           - f("heads_mean") * inv2).reshape(HN * CS)                     # [384]

    hw2, hb2 = f("heads_w2"), f("heads_b2")
    W3 = np.zeros((COUT, HN * CS), np.float32)                            # [12, 384]
    b3v = np.zeros((COUT,), np.float32)
    r = 0
    for h, ch in enumerate(HEAD_CH):
        W3[r : r + ch, h * CS : (h + 1) * CS] = hw2[h, :ch, :]
        b3v[r : r + ch] = hb2[h, :ch]
        r += ch

    # lhsT packings (lhsT = W.T, K-chunks of 128 side by side in the free dim)
    wp = np.zeros((128, W_COLS), np.float32)
    wp[:, W1_OFF : W1_OFF + 128] = (
        W1.T.reshape(2, 128, 64).transpose(1, 0, 2).reshape(128, 128))
    wp[0:64, W2_OFF : W2_OFF + 384] = W2.T
    wp[64:128, W2_OFF : W2_OFF + 384] = W2.T
    wp[:, W3_OFF : W3_OFF + 36] = (
        W3.T.reshape(3, 128, COUT).transpose(1, 0, 2).reshape(128, 36))

    bpk = np.zeros((128, B_COLS), np.float32)
    bpk[:CS, B1_OFF] = b1v
    bpk[CS : 2 * CS, B1_OFF] = b1v
    bpk[:, B2_OFF : B2_OFF + 3] = b2v.reshape(3, 128).T
    bpk[0:COUT, B3_OFF] = b3v
    bpk[32 : 32 + COUT, B3_OFF] = b3v

    wp = wp.astype(_np_mm_dtype(mm_dtype))

    return {"wp": wp, "bp": bpk}, b3v


def _make_in_maps(inputs, mm_dtype=MM_DTYPE):
    shared, _ = _fold_params(inputs, mm_dtype)
    ct = np.asarray(inputs["ct_feat"], np.float32)
    xdt = _np_mm_dtype(mm_dtype)
    return [{"x": np.ascontiguousarray(ct[b]).astype(xdt), **shared}
            for b in range(B)]


def _run(inputs, mm_dtype=MM_DTYPE, trace=False):
    from concourse.bass_utils import run_bass_kernel_spmd

    nc = _get_nc(mm_dtype)
    in_maps = _make_in_maps(inputs, mm_dtype)
    res = run_bass_kernel_spmd(nc, in_maps, core_ids=list(range(NCORES)),
                               trace=trace)
    out = np.stack([res.results[b]["out"] for b in range(B)], axis=0)
    return out, res


def kernel(**inputs) -> np.ndarray:
    out, _ = _run(inputs)
    return out
